# revision 11
# baseline (speedup 1.0000x reference)
"""Trainium2 Bass kernel for nn_LossBuilder_76553497084344.

reference:
    id_loss  = label-smoothed CE over pred_ids [4096, 8192], targets < 256
    dist     = euclidean dist of embeddings [4096, 2048] -> [4096, 4096]
    dist_ap  = rowmax over positives (same target id, incl diagonal)
    dist_an  = rowmin over negatives
    tri_loss = mean(relu(dist_ap - dist_an + 0.3))
    returns (id_loss, tri_loss, dist_ap, dist_an)

Device strategy (8 cores, no collectives — host shards / gathers):
  dist part, 4x2 grid (core c=(a,b): rows a*1024..+1024, cols b*2048..+2048):
    G[i,j] = sum_k stat[k,i] * mov[k,j] over K = 2048 emb rows (fp32r)
             + 256 one-hot class rows (bf16, +-128 -> -16384*eq[i,j])
             + 3 yy-feature rows (bf16 split of -yy/2 against ones)
           = xy[i,j] - 16384*eq[i,j] - yy[j]/2
    so -2G = yy[j] - 2 xy[i,j] + 32768*eq[i,j].  Row max/min of -2G over the
    col block give masked max-over-positives / min-over-negatives of
    (yy - 2xy) after the host adds xx and subtracts the 32768 bias
    (every row has >=1 positive: the diagonal; bias 32768 > max d2 ~ 6000).
    Device outputs rowmin(G), rowmax(G) (negation/scale folded in on host).
  CE part, row-sharded 512/core:
    device outputs rowsum(exp(logits)) and rowsum(logits); host does the
    target-logit gather, log, and means.
"""
import sys
if "/opt/trn_rl_repo" not in sys.path:
    sys.path.insert(0, "/opt/trn_rl_repo")

import numpy as np
import ml_dtypes

N, D, C = 4096, 2048, 8192
NUM_IDS = 256
MARGIN = np.float32(0.3)
EPS = np.float32(0.1)
CBIG = np.float32(32768.0)

N_CORES = 8
ROW_G, COL_G = 4, 2            # grid: 4 row-groups x 2 col-groups
DR = N // ROW_G                # 1024 dist rows per core
DC = N // COL_G                # 2048 dist cols per core
CE_R = N // N_CORES            # 512 CE rows per core
KT = D // 128                  # 16 fp32r k-tiles
NB = DC // 512                 # 4 moving col-blocks per core
MT = DR // 128                 # 8 output row-tiles per core
CE_MT = CE_R // 128            # 4 CE row-tiles
CE_CH = 4                      # CE column chunks per row-tile
CE_CW = C // CE_CH             # 2048

_compiled = None


def _build():
    import concourse.bacc as bacc
    import concourse.mybir as mybir
    import concourse.tile as tile

    nc = bacc.Bacc("TRN2", target_bir_lowering=False, debug=False)

    stat_d = nc.dram_tensor("stat", [KT, 128, DR], mybir.dt.float32r,
                            kind="ExternalInput").ap()
    mov_d = nc.dram_tensor("mov", [NB, KT, 128, 512], mybir.dt.float32r,
                           kind="ExternalInput").ap()
    astat_d = nc.dram_tensor("astat", [259, DR], mybir.dt.bfloat16,
                             kind="ExternalInput").ap()
    amov_d = nc.dram_tensor("amov", [259, DC], mybir.dt.bfloat16,
                            kind="ExternalInput").ap()
    pred_d = nc.dram_tensor("pred", [CE_R, C], mybir.dt.float32,
                            kind="ExternalInput").ap()

    # single batched output: cols m*4+n gmin block-partials, 32+m*4+n gmax,
    # 64+mt sexp, 68+mt slog; row p = partition (host folds the n blocks)
    outs_d = nc.dram_tensor("outs", [128, 72], mybir.dt.float32,
                            kind="ExternalOutput").ap()

    f32, f32r, bf16 = mybir.dt.float32, mybir.dt.float32r, mybir.dt.bfloat16
    Alu, Act = mybir.AluOpType, mybir.ActivationFunctionType

    with tile.TileContext(nc) as tc:
        with tc.tile_pool(name="stat_p", bufs=1, side="left") as stat_p, \
             tc.tile_pool(name="mov_p", bufs=2, side="right") as mov_p, \
             tc.tile_pool(name="ce_p", bufs=3, side="right") as ce_p, \
             tc.tile_pool(name="scr_p", bufs=2, side="left") as scr_p, \
             tc.tile_pool(name="stage_p", bufs=1, side="left") as stage_p, \
             tc.tile_pool(name="psum_p", bufs=8, space="PSUM") as psum_p:

            # ---- resident stationary: 16 f32r k-tiles + 3 aug tiles ----
            # (loads interleaved with the first moving block below so the
            #  first matmuls can start after ~1 MB instead of ~13 MB)
            stat_t = []
            for k in range(KT):
                st = stat_p.tile([128, DR], f32r, name=f"stat{k}", tag=f"stat{k}")
                stat_t.append(st)
            astat1 = stat_p.tile([128, DR], bf16, name="astat1", tag="astat1")
            astat2 = stat_p.tile([128, DR], bf16, name="astat2", tag="astat2")
            astat3 = stat_p.tile([3, DR], bf16, name="astat3", tag="astat3")

            out_sb = stage_p.tile([128, 72], f32, name="out_sb", tag="out_sb")

            def emit_ce_mtile(mt):
                sexp_stage = stage_p.tile([128, CE_CH], f32,
                                          name=f"sexp_stage{mt}",
                                          tag=f"sexp_stage{mt}")
                slog_stage = stage_p.tile([128, CE_CH], f32,
                                          name=f"slog_stage{mt}",
                                          tag=f"slog_stage{mt}")
                for ch in range(CE_CH):
                    lg = ce_p.tile([128, CE_CW], f32, name="lg", tag="lg")
                    nc.sync.dma_start(
                        lg[:], pred_d[mt * 128:(mt + 1) * 128,
                                      ch * CE_CW:(ch + 1) * CE_CW])
                    esc = scr_p.tile([128, CE_CW], f32, name="esc", tag="esc")
                    nc.scalar.activation(esc[:], lg[:], Act.Exp,
                                         accum_out=sexp_stage[:, ch:ch + 1])
                    ssc = scr_p.tile([128, CE_CW], f32, name="ssc", tag="ssc")
                    nc.vector.tensor_scalar(
                        out=ssc[:], in0=lg[:], scalar1=1.0, scalar2=0.0,
                        op0=Alu.mult, op1=Alu.add,
                        accum_out=slog_stage[:, ch:ch + 1])
                nc.vector.tensor_reduce(out_sb[:, 64 + mt:65 + mt],
                                        sexp_stage[:],
                                        axis=mybir.AxisListType.X, op=Alu.add)
                nc.vector.tensor_reduce(out_sb[:, 68 + mt:69 + mt],
                                        slog_stage[:],
                                        axis=mybir.AxisListType.X, op=Alu.add)

            # ---- dist: per col-block, stream moving operand, 8 m-tiles ----
            def load_mov_block(n):
                mov_t = []
                for k in range(KT):
                    mv = mov_p.tile([128, 512], f32r, name=f"mov{k}",
                                    tag=f"mov{k}")
                    nc.sync.dma_start(mv[:], mov_d[n, k])
                    mov_t.append(mv)
                    if n == 0:   # interleave resident stationary loads
                        nc.sync.dma_start(stat_t[k][:], stat_d[k])
                amov1 = mov_p.tile([128, 512], bf16, name="amov1", tag="amov1")
                amov2 = mov_p.tile([128, 512], bf16, name="amov2", tag="amov2")
                amov3 = mov_p.tile([3, 512], bf16, name="amov3", tag="amov3")
                nc.sync.dma_start(amov1[:], amov_d[0:128, n * 512:(n + 1) * 512])
                nc.sync.dma_start(amov2[:], amov_d[128:256, n * 512:(n + 1) * 512])
                nc.sync.dma_start(amov3[:], amov_d[256:259, n * 512:(n + 1) * 512])
                if n == 0:
                    nc.sync.dma_start(astat1[:], astat_d[0:128, :])
                    nc.sync.dma_start(astat2[:], astat_d[128:256, :])
                    nc.sync.dma_start(astat3[:], astat_d[256:259, :])
                return mov_t, amov1, amov2, amov3

            cur = load_mov_block(0)
            for n in range(NB):
                mov_t, amov1, amov2, amov3 = cur
                if n + 1 < NB:
                    nxt = load_mov_block(n + 1)   # prefetch next col-block
                emit_ce_mtile(n)                  # CE DMAs at lower priority

                for m in range(MT):
                    ms = slice(m * 128, (m + 1) * 128)
                    ps = psum_p.tile([128, 512], f32, name="ps", tag="ps")
                    for k in range(KT):
                        nc.tensor.matmul(ps[:], stat_t[k][:, ms], mov_t[k][:],
                                         start=(k == 0), stop=False)
                    nc.tensor.matmul(ps[:], astat1[:, ms], amov1[:],
                                     start=False, stop=False,
                                     skip_group_check=True)
                    nc.tensor.matmul(ps[:], astat2[:, ms], amov2[:],
                                     start=False, stop=False,
                                     skip_group_check=True)
                    nc.tensor.matmul(ps[:], astat3[:, ms], amov3[:],
                                     start=False, stop=True,
                                     skip_group_check=True)
                    nc.vector.tensor_reduce(out_sb[:, m * NB + n:m * NB + n + 1],
                                            ps[:],
                                            axis=mybir.AxisListType.X,
                                            op=Alu.min)
                    nc.vector.tensor_reduce(
                        out_sb[:, 32 + m * NB + n:33 + m * NB + n], ps[:],
                        axis=mybir.AxisListType.X, op=Alu.max)
                if n + 1 < NB:
                    cur = nxt

            nc.sync.dma_start(outs_d[:], out_sb[:])

    nc.compile()
    return nc


def _get_nc():
    global _compiled
    if _compiled is None:
        _compiled = _build()
    return _compiled


def _split3_bf16(x):
    """Split fp32 vector into 3 bf16 rows summing to ~x."""
    h1 = x.astype(ml_dtypes.bfloat16)
    r1 = x - h1.astype(np.float32)
    h2 = r1.astype(ml_dtypes.bfloat16)
    r2 = r1 - h2.astype(np.float32)
    h3 = r2.astype(ml_dtypes.bfloat16)
    return np.stack([h1, h2, h3])


def _prepare_in_maps(embeddings, pred_ids, target_ids):
    emb = np.ascontiguousarray(embeddings, dtype=np.float32)
    pred = np.ascontiguousarray(pred_ids, dtype=np.float32)
    tgt = np.asarray(target_ids).astype(np.int64)

    yy = np.einsum("nd,nd->n", emb, emb, dtype=np.float32)  # [N] row norms^2
    embT = np.ascontiguousarray(emb.T)                      # [D, N]

    onehotT = np.zeros((NUM_IDS, N), dtype=ml_dtypes.bfloat16)
    onehotT[tgt, np.arange(N)] = 128.0
    yy_rows = _split3_bf16((-0.5 * yy).astype(np.float32))  # [3, N] bf16

    astat_full = np.concatenate(
        [onehotT, np.ones((3, N), dtype=ml_dtypes.bfloat16)], axis=0)  # [259,N]
    amov_full = np.concatenate(
        [-onehotT, yy_rows.astype(ml_dtypes.bfloat16)], axis=0)        # [259,N]

    in_maps = []
    for c in range(N_CORES):
        a, b = c // COL_G, c % COL_G
        rsl = slice(a * DR, (a + 1) * DR)
        csl = slice(b * DC, (b + 1) * DC)
        stat = np.ascontiguousarray(
            embT[:, rsl].reshape(KT, 128, DR))
        movb = embT[:, csl]                                  # [D, DC]
        mov = np.ascontiguousarray(
            movb.reshape(D, NB, 512).transpose(1, 0, 2).reshape(NB, KT, 128, 512))
        in_maps.append({
            "stat": stat,
            "mov": mov,
            "astat": np.ascontiguousarray(astat_full[:, rsl]),
            "amov": np.ascontiguousarray(amov_full[:, csl]),
            "pred": np.ascontiguousarray(pred[c * CE_R:(c + 1) * CE_R]),
        })
    return in_maps, emb, pred, tgt, yy


def _postprocess(results, pred, tgt, yy):
    gmin = np.empty((ROW_G, COL_G, DR), np.float32)
    gmax = np.empty((ROW_G, COL_G, DR), np.float32)
    sexp = np.empty(N, np.float32)
    slog = np.empty(N, np.float32)
    for c in range(N_CORES):
        a, b = c // COL_G, c % COL_G
        o = results[c]["outs"]            # [128, 72]; [p, m*4+n] partials
        gmin[a, b] = o[:, 0:32].reshape(128, MT, NB).min(2).T.reshape(DR)
        gmax[a, b] = o[:, 32:64].reshape(128, MT, NB).max(2).T.reshape(DR)
        sexp[c * CE_R:(c + 1) * CE_R] = o[:, 64:68].T.reshape(CE_R)
        slog[c * CE_R:(c + 1) * CE_R] = o[:, 68:72].T.reshape(CE_R)

    # rowmax(V) over all cols = max over col-groups of (-2 * blockmin(G))
    vmax = (-2.0 * gmin).max(axis=1).reshape(N)   # [ROW_G*DR] = [N], row order
    vmin = (-2.0 * gmax).min(axis=1).reshape(N)
    xx = yy  # same vector, indexed by row
    d2ap = (vmax - CBIG + xx).astype(np.float32)
    d2an = (vmin + xx).astype(np.float32)
    dist_ap = np.sqrt(np.maximum(d2ap, np.float32(1e-12)), dtype=np.float32)
    dist_an = np.sqrt(np.maximum(d2an, np.float32(1e-12)), dtype=np.float32)
    tri_loss = np.float32(
        np.mean(np.maximum(dist_ap - dist_an + MARGIN, np.float32(0.0)),
                dtype=np.float32))

    lse = np.log(sexp, dtype=np.float32)
    tgt_logit = pred[np.arange(N), tgt]
    nll = lse - tgt_logit
    smooth = lse - slog / np.float32(C)
    id_loss = np.float32(
        np.mean((np.float32(1.0) - EPS) * nll + EPS * smooth, dtype=np.float32))

    return id_loss, tri_loss, dist_ap, dist_an


def _run(inputs, trace=False, trace_cores=None):
    from concourse.bass_utils import run_bass_kernel_spmd
    in_maps, emb, pred, tgt, yy = _prepare_in_maps(**inputs)
    nc = _get_nc()
    res = run_bass_kernel_spmd(nc, in_maps, core_ids=list(range(N_CORES)),
                               trace=trace,
                               **({"trace_cores": trace_cores}
                                  if trace_cores else {}))
    out = _postprocess(res.results, pred, tgt, yy)
    return out, res


def kernel(embeddings, pred_ids, target_ids):
    out, _ = _run({"embeddings": embeddings, "pred_ids": pred_ids,
                   "target_ids": target_ids})
    return out


# revision 12
# speedup vs baseline: 1.0270x; 1.0270x over previous
"""Trainium2 Bass kernel for nn_LossBuilder_76553497084344.

reference:
    id_loss  = label-smoothed CE over pred_ids [4096, 8192], targets < 256
    dist     = euclidean dist of embeddings [4096, 2048] -> [4096, 4096]
    dist_ap  = rowmax over positives (same target id, incl diagonal)
    dist_an  = rowmin over negatives
    tri_loss = mean(relu(dist_ap - dist_an + 0.3))
    returns (id_loss, tri_loss, dist_ap, dist_an)

Device strategy (8 cores, no collectives — host shards / gathers):
  dist part, 4x2 grid (core c=(a,b): rows a*1024..+1024, cols b*2048..+2048):
    G[i,j] = sum_k stat[k,i] * mov[k,j] over K = 2048 emb rows (fp32r)
             + 256 one-hot class rows (bf16, +-128 -> -16384*eq[i,j])
             + 3 yy-feature rows (bf16 split of -yy/2 against ones)
           = xy[i,j] - 16384*eq[i,j] - yy[j]/2
    so -2G = yy[j] - 2 xy[i,j] + 32768*eq[i,j].  Row max/min of -2G over the
    col block give masked max-over-positives / min-over-negatives of
    (yy - 2xy) after the host adds xx and subtracts the 32768 bias
    (every row has >=1 positive: the diagonal; bias 32768 > max d2 ~ 6000).
    Device outputs rowmin(G), rowmax(G) (negation/scale folded in on host).
  CE part, row-sharded 512/core:
    device outputs rowsum(exp(logits)) and rowsum(logits); host does the
    target-logit gather, log, and means.
"""
import sys
if "/opt/trn_rl_repo" not in sys.path:
    sys.path.insert(0, "/opt/trn_rl_repo")

import numpy as np
import ml_dtypes

N, D, C = 4096, 2048, 8192
NUM_IDS = 256
MARGIN = np.float32(0.3)
EPS = np.float32(0.1)
CBIG = np.float32(32768.0)

N_CORES = 8
ROW_G, COL_G = 4, 2            # grid: 4 row-groups x 2 col-groups
DR = N // ROW_G                # 1024 dist rows per core
DC = N // COL_G                # 2048 dist cols per core
CE_R = N // N_CORES            # 512 CE rows per core
KT = D // 128                  # 16 fp32r k-tiles
NB = DC // 512                 # 4 moving col-blocks per core
MT = DR // 128                 # 8 output row-tiles per core
CE_MT = CE_R // 128            # 4 CE row-tiles
CE_CH = 4                      # CE column chunks per row-tile
CE_CW = C // CE_CH             # 2048

_compiled = None


def _build():
    import concourse.bacc as bacc
    import concourse.mybir as mybir
    import concourse.tile as tile

    nc = bacc.Bacc("TRN2", target_bir_lowering=False, debug=False)

    stat_d = nc.dram_tensor("stat", [KT, 128, DR], mybir.dt.float32r,
                            kind="ExternalInput").ap()
    mov_d = nc.dram_tensor("mov", [NB, KT, 128, 512], mybir.dt.float32r,
                           kind="ExternalInput").ap()
    astat_d = nc.dram_tensor("astat", [259, DR], mybir.dt.bfloat16,
                             kind="ExternalInput").ap()
    amov_d = nc.dram_tensor("amov", [259, DC], mybir.dt.bfloat16,
                            kind="ExternalInput").ap()
    pred_d = nc.dram_tensor("pred", [CE_R, C], mybir.dt.float32,
                            kind="ExternalInput").ap()

    # single batched output: cols m*4+n gmin block-partials, 32+m*4+n gmax,
    # 64+mt sexp, 68+mt slog; row p = partition (host folds the n blocks)
    outs_d = nc.dram_tensor("outs", [128, 72], mybir.dt.float32,
                            kind="ExternalOutput").ap()

    f32, f32r, bf16 = mybir.dt.float32, mybir.dt.float32r, mybir.dt.bfloat16
    Alu, Act = mybir.AluOpType, mybir.ActivationFunctionType

    with tile.TileContext(nc) as tc:
        with tc.tile_pool(name="stat_p", bufs=1, side="left") as stat_p, \
             tc.tile_pool(name="mov_p", bufs=2, side="right") as mov_p, \
             tc.tile_pool(name="ce_p", bufs=3, side="right") as ce_p, \
             tc.tile_pool(name="scr_p", bufs=2, side="left") as scr_p, \
             tc.tile_pool(name="stage_p", bufs=1, side="left") as stage_p, \
             tc.tile_pool(name="psum_p", bufs=8, space="PSUM") as psum_p:

            # ---- resident stationary: 16 f32r k-tiles + 3 aug tiles ----
            # (loads interleaved with the first moving block below so the
            #  first matmuls can start after ~1 MB instead of ~13 MB)
            stat_t = []
            for k in range(KT):
                st = stat_p.tile([128, DR], f32r, name=f"stat{k}", tag=f"stat{k}")
                stat_t.append(st)
            astat1 = stat_p.tile([128, DR], bf16, name="astat1", tag="astat1")
            astat2 = stat_p.tile([128, DR], bf16, name="astat2", tag="astat2")
            astat3 = stat_p.tile([3, DR], bf16, name="astat3", tag="astat3")

            out_sb = stage_p.tile([128, 72], f32, name="out_sb", tag="out_sb")

            def emit_ce_mtile(mt):
                sexp_stage = stage_p.tile([128, CE_CH], f32,
                                          name=f"sexp_stage{mt}",
                                          tag=f"sexp_stage{mt}")
                slog_stage = stage_p.tile([128, CE_CH], f32,
                                          name=f"slog_stage{mt}",
                                          tag=f"slog_stage{mt}")
                for ch in range(CE_CH):
                    lg = ce_p.tile([128, CE_CW], f32, name="lg", tag="lg")
                    nc.sync.dma_start(
                        lg[:], pred_d[mt * 128:(mt + 1) * 128,
                                      ch * CE_CW:(ch + 1) * CE_CW])
                    esc = scr_p.tile([128, CE_CW], f32, name="esc", tag="esc")
                    nc.scalar.activation(esc[:], lg[:], Act.Exp,
                                         accum_out=sexp_stage[:, ch:ch + 1])
                    ssc = scr_p.tile([128, CE_CW], f32, name="ssc", tag="ssc")
                    nc.scalar.activation(ssc[:], lg[:], Act.Identity,
                                         accum_out=slog_stage[:, ch:ch + 1])
                nc.vector.tensor_reduce(out_sb[:, 64 + mt:65 + mt],
                                        sexp_stage[:],
                                        axis=mybir.AxisListType.X, op=Alu.add)
                nc.vector.tensor_reduce(out_sb[:, 68 + mt:69 + mt],
                                        slog_stage[:],
                                        axis=mybir.AxisListType.X, op=Alu.add)

            # ---- dist: per col-block, stream moving operand, 8 m-tiles ----
            def load_mov_block(n):
                mov_t = []
                for k in range(KT):
                    mv = mov_p.tile([128, 512], f32r, name=f"mov{k}",
                                    tag=f"mov{k}")
                    nc.sync.dma_start(mv[:], mov_d[n, k])
                    mov_t.append(mv)
                    if n == 0:   # interleave resident stationary loads
                        nc.sync.dma_start(stat_t[k][:], stat_d[k])
                amov1 = mov_p.tile([128, 512], bf16, name="amov1", tag="amov1")
                amov2 = mov_p.tile([128, 512], bf16, name="amov2", tag="amov2")
                amov3 = mov_p.tile([3, 512], bf16, name="amov3", tag="amov3")
                nc.sync.dma_start(amov1[:], amov_d[0:128, n * 512:(n + 1) * 512])
                nc.sync.dma_start(amov2[:], amov_d[128:256, n * 512:(n + 1) * 512])
                nc.sync.dma_start(amov3[:], amov_d[256:259, n * 512:(n + 1) * 512])
                if n == 0:
                    nc.sync.dma_start(astat1[:], astat_d[0:128, :])
                    nc.sync.dma_start(astat2[:], astat_d[128:256, :])
                    nc.sync.dma_start(astat3[:], astat_d[256:259, :])
                return mov_t, amov1, amov2, amov3

            cur = load_mov_block(0)
            for n in range(NB):
                mov_t, amov1, amov2, amov3 = cur
                if n + 1 < NB:
                    nxt = load_mov_block(n + 1)   # prefetch next col-block
                emit_ce_mtile(n)                  # CE DMAs at lower priority

                for m in range(MT):
                    ms = slice(m * 128, (m + 1) * 128)
                    ps = psum_p.tile([128, 512], f32, name="ps", tag="ps")
                    for k in range(KT):
                        nc.tensor.matmul(ps[:], stat_t[k][:, ms], mov_t[k][:],
                                         start=(k == 0), stop=False)
                    nc.tensor.matmul(ps[:], astat1[:, ms], amov1[:],
                                     start=False, stop=False,
                                     skip_group_check=True)
                    nc.tensor.matmul(ps[:], astat2[:, ms], amov2[:],
                                     start=False, stop=False,
                                     skip_group_check=True)
                    nc.tensor.matmul(ps[:], astat3[:, ms], amov3[:],
                                     start=False, stop=True,
                                     skip_group_check=True)
                    nc.vector.tensor_reduce(out_sb[:, m * NB + n:m * NB + n + 1],
                                            ps[:],
                                            axis=mybir.AxisListType.X,
                                            op=Alu.min)
                    nc.vector.tensor_reduce(
                        out_sb[:, 32 + m * NB + n:33 + m * NB + n], ps[:],
                        axis=mybir.AxisListType.X, op=Alu.max)
                if n + 1 < NB:
                    cur = nxt

            nc.sync.dma_start(outs_d[:], out_sb[:])

    nc.compile()
    return nc


def _get_nc():
    global _compiled
    if _compiled is None:
        _compiled = _build()
    return _compiled


def _split3_bf16(x):
    """Split fp32 vector into 3 bf16 rows summing to ~x."""
    h1 = x.astype(ml_dtypes.bfloat16)
    r1 = x - h1.astype(np.float32)
    h2 = r1.astype(ml_dtypes.bfloat16)
    r2 = r1 - h2.astype(np.float32)
    h3 = r2.astype(ml_dtypes.bfloat16)
    return np.stack([h1, h2, h3])


def _prepare_in_maps(embeddings, pred_ids, target_ids):
    emb = np.ascontiguousarray(embeddings, dtype=np.float32)
    pred = np.ascontiguousarray(pred_ids, dtype=np.float32)
    tgt = np.asarray(target_ids).astype(np.int64)

    yy = np.einsum("nd,nd->n", emb, emb, dtype=np.float32)  # [N] row norms^2
    embT = np.ascontiguousarray(emb.T)                      # [D, N]

    onehotT = np.zeros((NUM_IDS, N), dtype=ml_dtypes.bfloat16)
    onehotT[tgt, np.arange(N)] = 128.0
    yy_rows = _split3_bf16((-0.5 * yy).astype(np.float32))  # [3, N] bf16

    astat_full = np.concatenate(
        [onehotT, np.ones((3, N), dtype=ml_dtypes.bfloat16)], axis=0)  # [259,N]
    amov_full = np.concatenate(
        [-onehotT, yy_rows.astype(ml_dtypes.bfloat16)], axis=0)        # [259,N]

    in_maps = []
    for c in range(N_CORES):
        a, b = c // COL_G, c % COL_G
        rsl = slice(a * DR, (a + 1) * DR)
        csl = slice(b * DC, (b + 1) * DC)
        stat = np.ascontiguousarray(
            embT[:, rsl].reshape(KT, 128, DR))
        movb = embT[:, csl]                                  # [D, DC]
        mov = np.ascontiguousarray(
            movb.reshape(D, NB, 512).transpose(1, 0, 2).reshape(NB, KT, 128, 512))
        in_maps.append({
            "stat": stat,
            "mov": mov,
            "astat": np.ascontiguousarray(astat_full[:, rsl]),
            "amov": np.ascontiguousarray(amov_full[:, csl]),
            "pred": np.ascontiguousarray(pred[c * CE_R:(c + 1) * CE_R]),
        })
    return in_maps, emb, pred, tgt, yy


def _postprocess(results, pred, tgt, yy):
    gmin = np.empty((ROW_G, COL_G, DR), np.float32)
    gmax = np.empty((ROW_G, COL_G, DR), np.float32)
    sexp = np.empty(N, np.float32)
    slog = np.empty(N, np.float32)
    for c in range(N_CORES):
        a, b = c // COL_G, c % COL_G
        o = results[c]["outs"]            # [128, 72]; [p, m*4+n] partials
        gmin[a, b] = o[:, 0:32].reshape(128, MT, NB).min(2).T.reshape(DR)
        gmax[a, b] = o[:, 32:64].reshape(128, MT, NB).max(2).T.reshape(DR)
        sexp[c * CE_R:(c + 1) * CE_R] = o[:, 64:68].T.reshape(CE_R)
        slog[c * CE_R:(c + 1) * CE_R] = o[:, 68:72].T.reshape(CE_R)

    # rowmax(V) over all cols = max over col-groups of (-2 * blockmin(G))
    vmax = (-2.0 * gmin).max(axis=1).reshape(N)   # [ROW_G*DR] = [N], row order
    vmin = (-2.0 * gmax).min(axis=1).reshape(N)
    xx = yy  # same vector, indexed by row
    d2ap = (vmax - CBIG + xx).astype(np.float32)
    d2an = (vmin + xx).astype(np.float32)
    dist_ap = np.sqrt(np.maximum(d2ap, np.float32(1e-12)), dtype=np.float32)
    dist_an = np.sqrt(np.maximum(d2an, np.float32(1e-12)), dtype=np.float32)
    tri_loss = np.float32(
        np.mean(np.maximum(dist_ap - dist_an + MARGIN, np.float32(0.0)),
                dtype=np.float32))

    lse = np.log(sexp, dtype=np.float32)
    tgt_logit = pred[np.arange(N), tgt]
    nll = lse - tgt_logit
    smooth = lse - slog / np.float32(C)
    id_loss = np.float32(
        np.mean((np.float32(1.0) - EPS) * nll + EPS * smooth, dtype=np.float32))

    return id_loss, tri_loss, dist_ap, dist_an


def _run(inputs, trace=False, trace_cores=None):
    from concourse.bass_utils import run_bass_kernel_spmd
    in_maps, emb, pred, tgt, yy = _prepare_in_maps(**inputs)
    nc = _get_nc()
    res = run_bass_kernel_spmd(nc, in_maps, core_ids=list(range(N_CORES)),
                               trace=trace,
                               **({"trace_cores": trace_cores}
                                  if trace_cores else {}))
    out = _postprocess(res.results, pred, tgt, yy)
    return out, res


def kernel(embeddings, pred_ids, target_ids):
    out, _ = _run({"embeddings": embeddings, "pred_ids": pred_ids,
                   "target_ids": target_ids})
    return out
